# revision 1
# baseline (speedup 1.0000x reference)
"""Multi-head attention (B=16, C=256, N=1024, H=4 heads) on 8 TRN2 NeuronCores.

Data-parallel over batch: 2 images per core, weights replicated, no
collectives. All GEMMs run in bf16 with fp32 PSUM accumulation (simulated
end-to-end rel err ~5e-4); softmax statistics, normalization and the
residual path stay fp32.

Layout strategy: everything stays "transposed" ([feature, token]) so the
whole chain — qk projection, scores, AV, out projection — needs zero
on-chip transposes:
  qkT[3C', N]  = W_proj_slices.T @ x_r          (lhsT = W_proj, rhs = x natural)
  attT[j, i]   = k @ q.T                        (lhsT = kT cols, rhs = qT)
  E            = exp(attT * scale)              (ScalarE, PSUM -> SBUF, bf16)
  outT[d, i]   = v.T @ E  (lhsT = v natural)    + ones-lhsT matmul -> denominator
  resT[c, i]   = W_out.T @ concatT + bias + x_r (exact output DRAM layout)
The softmax denominator comes from a [128,128] ones lhsT matmul over E's
j-tiles: every PSUM partition row holds s[i], i.e. already broadcast.

Scheduling notes (measured on HW):
 - DMAs ordered so the first head's weights + x land first; dummy bf16
   warmup matmuls bridge the initial DMA wait and keep the PE clock-gate
   (HAM) warm so real matmuls start at 2.4 GHz.
 - PSUM->SBUF copies ride the ScalarEngine; the DVE is kept nearly
   dedicated to the softmax drain (reciprocal_approx_fast + normalize
   muls) so AV accumulator banks recycle fast.
 - Weights/x are DMA'd as fp32 and cast to bf16 on-chip (DMA cannot
   convert dtypes).
"""
import sys

try:
    import concourse.bass as bass  # noqa: F401
except ImportError:
    sys.path.insert(0, "/opt/trn_rl_repo")

from contextlib import ExitStack

import numpy as np

import concourse.bass as bass
import concourse.mybir as mybir
import concourse.tile as tile
from concourse import bacc
from concourse.bass_utils import run_bass_kernel_spmd

F32 = mybir.dt.float32
BF16 = mybir.dt.bfloat16
FP8 = mybir.dt.float8e5
EXP = mybir.ActivationFunctionType.Exp
IDENT = mybir.ActivationFunctionType.Identity

B_PER_CORE = 2   # 16 images / 8 cores
C = 256          # channels == head dim
N = 1024         # tokens (32*32)
HEADS = 4
SCALE = C ** -0.5
N_CORES = 8


def _build():
    nc = bacc.Bacc("TRN2", debug=False, num_devices=N_CORES)
    x_d = nc.declare_dram_parameter("x", [B_PER_CORE, C, N], F32, isOutput=False)
    wp_d = nc.declare_dram_parameter("W_proj", [C, 3 * HEADS * C], F32, isOutput=False)
    bp_d = nc.declare_dram_parameter("b_proj", [3 * HEADS * C], F32, isOutput=False)
    wo_d = nc.declare_dram_parameter("W_out", [HEADS * C, C], F32, isOutput=False)
    bo_d = nc.declare_dram_parameter("b_out", [C], F32, isOutput=False)
    out_d = nc.declare_dram_parameter("out", [B_PER_CORE, C, N], F32, isOutput=True)

    with tile.TileContext(nc) as tc, ExitStack() as ctx:
        pool = ctx.enter_context(tc.tile_pool(name="persist", bufs=1))
        stage_pool = ctx.enter_context(tc.tile_pool(name="stage", bufs=3))
        xr_pool = ctx.enter_context(tc.tile_pool(name="xr", bufs=2))
        xb_pool = ctx.enter_context(tc.tile_pool(name="xb", bufs=2))
        v2_pool = ctx.enter_context(tc.tile_pool(name="v2", bufs=1))
        qk_pool = ctx.enter_context(tc.tile_pool(name="qk", bufs=2))
        e_pool = ctx.enter_context(tc.tile_pool(name="e", bufs=2))
        e8_pool = ctx.enter_context(tc.tile_pool(name="e8", bufs=2))
        cat_pool = ctx.enter_context(tc.tile_pool(name="cat", bufs=1))
        r_pool = ctx.enter_context(tc.tile_pool(name="r", bufs=2))
        xrb_pool = ctx.enter_context(tc.tile_pool(name="xrb", bufs=2))
        out_pool = ctx.enter_context(tc.tile_pool(name="outs", bufs=4))
        ps_work = ctx.enter_context(tc.tile_pool(name="psw", bufs=5, space="PSUM"))
        ps_acc = ctx.enter_context(tc.tile_pool(name="psa", bufs=2, space="PSUM"))
        ps_s = ctx.enter_context(tc.tile_pool(name="pss", bufs=1, space="PSUM"))

        # ---- DMAs + on-chip bf16 casts, first-needed data first ----
        xr_tiles = []
        xr = xr_pool.tile([128, 2, N], F32, tag="xr")
        for kt in range(2):
            for isl in range(2):
                nc.sync.dma_start(
                    out=xr[:, kt, isl * 512:(isl + 1) * 512],
                    in_=x_d[0, kt * 128:(kt + 1) * 128, isl * 512:(isl + 1) * 512])
        xr_tiles.append(xr)

        w_sb = pool.tile([128, 2, 3072], BF16)  # W_proj k-tiles, per-head chunks
        b_sb = None
        for h in range(HEADS):
            for kt in range(2):
                ws = stage_pool.tile([128, 768], F32, tag="wstage")
                nc.sync.dma_start(
                    out=ws[:],
                    in_=wp_d[kt * 128:(kt + 1) * 128, h * 768:(h + 1) * 768])
                nc.vector.tensor_copy(w_sb[:, kt, h * 768:(h + 1) * 768], ws[:])
            if h == 0:
                # biases: needed by the first qk PSUM->SBUF copy, not the MMs
                b_sb = pool.tile([128, 24], F32)  # b_proj, tile t
                nc.sync.dma_start(
                    out=b_sb[:], in_=bp_d[:].rearrange("(t p) -> p t", p=128))
                bo_sb = pool.tile([128, 2], F32)
                nc.sync.dma_start(out=bo_sb[:],
                                  in_=bo_d[:].rearrange("(t p) -> p t", p=128))

        # second image's x: queued last, prefetched during image-0 compute
        xr = xr_pool.tile([128, 2, N], F32, tag="xr")
        for kt in range(2):
            nc.sync.dma_start(out=xr[:, kt, :],
                              in_=x_d[1, kt * 128:(kt + 1) * 128, :])
        xr_tiles.append(xr)

        # ---- small constants ----
        ones_f = pool.tile([128, 512], F32)
        nc.vector.memset(ones_f[:], 1.0)
        ones_w = pool.tile([128, 512], BF16)
        nc.vector.tensor_copy(ones_w[:], ones_f[:])
        ones_sb = ones_w[:, 0:128]
        ones8 = pool.tile([128, 2, 128], FP8)
        nc.vector.tensor_copy(ones8[:],
                              ones_f[:, 0:256].rearrange("p (a b) -> p a b", b=128))

        # dummy matmuls: fill the initial DMA wait + warm the HAM clock gate
        for wi in range(20):
            warm_ps = ps_work.tile([128, 512], F32, tag="work")
            nc.tensor.matmul(out=warm_ps[:], lhsT=ones_sb, rhs=ones_w[:],
                             start=True, stop=True)

        total_bias = pool.tile([128, 2], F32)
        wo_sb = pool.tile([128, 8, 256], BF16)  # W_out k-tiles (loaded mid-image-0)
        zb = pool.tile([128, 8, 2], BF16)

        def qk_proj(xb, h):
            """q,k for head h -> [128, 4(q0 q1 k0 k1), N] bf16."""
            qk = qk_pool.tile([128, 4, N], BF16, tag="qk")
            for mt in range(4):
                cols = h * 768 + mt * 128
                ps0 = ps_work.tile([128, 512], F32, tag="work")
                ps1 = ps_work.tile([128, 512], F32, tag="work")
                ps = [ps0, ps1]
                for kt in range(2):
                    for isl in range(2):
                        nc.tensor.matmul(
                            out=ps[isl][:],
                            lhsT=w_sb[:, kt, cols:cols + 128],
                            rhs=xb[:, kt, isl * 512:(isl + 1) * 512],
                            start=(kt == 0), stop=(kt == 1))
                for isl in range(2):
                    nc.scalar.activation(qk[:, mt, isl * 512:(isl + 1) * 512],
                                         ps[isl][:], IDENT,
                                         bias=b_sb[:, h * 6 + mt:h * 6 + mt + 1])
            return qk

        def v_proj(xb, v2, hp):
            """v for heads 2hp, 2hp+1 -> v2[:, it, h*256+d] (natural layout)."""
            for it in range(8):
                ps = ps_work.tile([128, 512], F32, tag="work")
                for kt in range(2):
                    rhs = w_sb[:, kt, :].rearrange(
                        "p (h c) -> p h c", h=HEADS
                    )[:, 2 * hp:2 * hp + 2, 512:768]
                    nc.tensor.matmul(out=ps[:],
                                     lhsT=xb[:, kt, it * 128:(it + 1) * 128],
                                     rhs=rhs, start=(kt == 0), stop=(kt == 1))
                nc.scalar.copy(v2[:, it, hp * 512:(hp + 1) * 512], ps[:])

        def attT_e(qk):
            """scores attT[j, i] -> E = exp(attT * scale) (+ fp8 shadow for s)."""
            e_t = e_pool.tile([128, 8, N], BF16, tag="e")
            e8 = e8_pool.tile([128, 2, 8, 512], FP8, tag="e8")
            for isl in range(2):
                for jt in range(8):
                    ps = ps_work.tile([128, 512], F32, tag="work")
                    for dt in range(2):
                        nc.tensor.matmul(
                            out=ps[:],
                            lhsT=qk[:, 2 + dt, jt * 128:(jt + 1) * 128],
                            rhs=qk[:, dt, isl * 512:(isl + 1) * 512],
                            start=(dt == 0), stop=(dt == 1))
                    nc.scalar.activation(e_t[:, jt, isl * 512:(isl + 1) * 512],
                                         ps[:], EXP, scale=SCALE)
                    nc.vector.tensor_scalar_mul(
                        e8[:, isl, jt, :],
                        e_t[:, jt, isl * 512:(isl + 1) * 512], 0.0625)
            return e_t, e8

        def av_isl(e_t, e8, v2, cat, h, isl):
            """AV + denominator for one i-half; normalized into concatT.
            The denominator sums fp8 E at DoubleRow half-rate (4 matmuls
            contract 256 j each: j = 256a + p + 128*pair)."""
            o_ps0 = ps_acc.tile([128, 512], F32, tag="acc")
            o_ps1 = ps_acc.tile([128, 512], F32, tag="acc")
            s_ps = ps_s.tile([128, 512], F32, tag="sacc")
            for jt in range(8):
                e_ap = e_t[:, jt, isl * 512:(isl + 1) * 512]
                st, sp = (jt == 0), (jt == 7)
                nc.tensor.matmul(out=o_ps0[:], rhs=e_ap, start=st, stop=sp,
                                 lhsT=v2[:, jt, h * 256:h * 256 + 128])
                nc.tensor.matmul(out=o_ps1[:], rhs=e_ap, start=st, stop=sp,
                                 lhsT=v2[:, jt, h * 256 + 128:h * 256 + 256])
            for a in range(4):
                nc.tensor.matmul(
                    out=s_ps[:], lhsT=ones8[:],
                    rhs=e8[:, isl, 2 * a:2 * a + 2, :],
                    perf_mode=mybir.MatmulPerfMode.DoubleRow,
                    start=(a == 0), stop=(a == 3))
            r_sb = r_pool.tile([128, 512], F32, tag="r")
            nc.vector.reciprocal_approx_fast(r_sb[:], s_ps[:])
            MUL = mybir.AluOpType.mult
            nc.vector.scalar_tensor_tensor(
                cat[:, 2 * h, isl * 512:(isl + 1) * 512], o_ps0[:], 0.0625,
                r_sb[:], MUL, MUL)
            nc.vector.scalar_tensor_tensor(
                cat[:, 2 * h + 1, isl * 512:(isl + 1) * 512], o_ps1[:], 0.0625,
                r_sb[:], MUL, MUL)

        for b in range(B_PER_CORE):
            xr = xr_tiles[b]
            xb = xb_pool.tile([128, 2, N], BF16, tag="xb")
            nc.scalar.copy(xb[:], xr[:])
            v2 = v2_pool.tile([128, 8, 1024], BF16, tag="v2")
            cat = cat_pool.tile([128, 8, N], BF16, tag="cat")

            qk = qk_proj(xb, 0)
            v_proj(xb, v2, 0)
            e_t, e8 = attT_e(qk)
            av_isl(e_t, e8, v2, cat, 0, 0)
            av_isl(e_t, e8, v2, cat, 0, 1)
            qk = qk_proj(xb, 1)
            e_t, e8 = attT_e(qk)
            av_isl(e_t, e8, v2, cat, 1, 0)
            av_isl(e_t, e8, v2, cat, 1, 1)
            if b == 0:
                for kt in range(8):
                    ws = stage_pool.tile([128, 256], F32, tag="wostage")
                    nc.sync.dma_start(out=ws[:],
                                      in_=wo_d[kt * 128:(kt + 1) * 128, :])
                    nc.vector.tensor_copy(wo_sb[:, kt, :], ws[:])
                zscr = stage_pool.tile([128, 16], F32, tag="zscr")
                nc.vector.memset(zscr[:], 0.0)
                nc.vector.tensor_copy(zb[:],
                                      zscr[:].rearrange("p (a b) -> p a b", b=2))
                for kt in range(8):
                    hh, dt = kt // 2, kt % 2
                    nc.vector.tensor_copy(
                        zb[:, kt, 0:1],
                        b_sb[:, hh * 6 + 4 + dt:hh * 6 + 5 + dt])

            qk = qk_proj(xb, 2)
            v_proj(xb, v2, 1)
            e_t, e8 = attT_e(qk)
            av_isl(e_t, e8, v2, cat, 2, 0)
            av_isl(e_t, e8, v2, cat, 2, 1)
            qk = qk_proj(xb, 3)
            e_t, e8 = attT_e(qk)
            av_isl(e_t, e8, v2, cat, 3, 0)
            av_isl(e_t, e8, v2, cat, 3, 1)

            if b == 0:
                # b_v folds through softmax (weights sum to 1) and W_out:
                # total_bias[c] = b_out[c] + sum_hd b_v[hd] * W_out[hd, c].
                # Deferred here so it doesn't stall the PE on the W_out DMA.
                for ct in range(2):
                    bias_ps = ps_work.tile([128, 2], F32, tag="work")
                    for kt in range(8):
                        nc.tensor.matmul(out=bias_ps[:],
                                         lhsT=wo_sb[:, kt, ct * 128:(ct + 1) * 128],
                                         rhs=zb[:, kt, :],
                                         start=(kt == 0), stop=(kt == 7))
                    nc.vector.tensor_add(total_bias[:, ct:ct + 1], bias_ps[:, 0:1],
                                         bo_sb[:, ct:ct + 1])

            # residual + bias, broadcast along tokens: xrb = x_r + total_bias
            xrb = xrb_pool.tile([128, 2, N], F32, tag="xrb")
            for ct in range(2):
                nc.scalar.activation(xrb[:, ct, :], xr[:, ct, :],
                                     IDENT, bias=total_bias[:, ct:ct + 1])

            # ---- out projection + residual, already in output layout ----
            for ct in range(2):
                for isl in range(2):
                    res_ps = ps_work.tile([128, 512], F32, tag="work")
                    for kt in range(8):
                        nc.tensor.matmul(
                            out=res_ps[:],
                            lhsT=wo_sb[:, kt, ct * 128:(ct + 1) * 128],
                            rhs=cat[:, kt, isl * 512:(isl + 1) * 512],
                            start=(kt == 0), stop=(kt == 7))
                    o_sb = out_pool.tile([128, 512], F32, tag="o_sb")
                    nc.vector.tensor_add(o_sb[:], res_ps[:],
                                         xrb[:, ct, isl * 512:(isl + 1) * 512])
                    nc.sync.dma_start(
                        out=out_d[b, ct * 128:(ct + 1) * 128,
                                  isl * 512:(isl + 1) * 512],
                        in_=o_sb[:])

    nc.compile()
    return nc


_NC = None


def kernel(x, W_proj, b_proj, W_out, b_out):
    global _NC
    if _NC is None:
        _NC = _build()
    x = np.ascontiguousarray(x, dtype=np.float32).reshape(16, C, N)
    in_maps = [
        {
            "x": x[i * B_PER_CORE:(i + 1) * B_PER_CORE],
            "W_proj": np.ascontiguousarray(W_proj, dtype=np.float32),
            "b_proj": np.ascontiguousarray(b_proj, dtype=np.float32),
            "W_out": np.ascontiguousarray(W_out, dtype=np.float32),
            "b_out": np.ascontiguousarray(b_out, dtype=np.float32),
        }
        for i in range(N_CORES)
    ]
    res = run_bass_kernel_spmd(_NC, in_maps, core_ids=list(range(N_CORES)))
    out = np.concatenate([res.results[i]["out"] for i in range(N_CORES)], axis=0)
    return out.reshape(16, C, 32, 32)



# revision 7
# speedup vs baseline: 1.5431x; 1.5431x over previous
"""Multi-head attention (B=16, C=256, N=1024, H=4 heads) on 8 TRN2 NeuronCores.

Data-parallel over batch: 2 images per core, weights replicated, no
collectives.

v2: every GEMM runs in fp8e4m3 with DoubleRow perf mode. Trace analysis of
the bf16 version showed MATMUL issue-to-issue spacing is ~259ns for N=512
regardless of dtype, and fp8 DR contracts 256 rows per slot vs 128 for
bf16 — exactly 2x FLOPs per slot (LDWEIGHTS ~162ns rides fully hidden).
This halves PE busy time from ~220us to ~116us per core. Simulated
end-to-end rel err ~8e-3 (gate 2e-2).

Layouts are all "transposed" ([feature, token]) as before — zero on-chip
transposes. fp8 packing for DoubleRow: both operands are [128, 2, X] APs
where the contraction index is (partition + 128*plane):
  qkT[4mt, N]   = W_qk.T @ x          (lhsT = W k-planes, rhs = xb k-planes)
  attT[j, i]    = k @ q.T             (lhsT = kT d-planes, rhs = qT d-planes)
  E'            = exp(attT/16)*2^-6   (ScalarE, PSUM -> SBUF fp8; 2^-6 keeps
                                       maxE ~48 << 240 fp8e4 sat limit)
  outT[d, i]    = v.T @ E'            (lhsT = v jt-planes)
  s[i]          = ones.T @ E'         (128-row broadcast denominator)
  resT[c, i]    = W_out.T @ catT + x_r + bias
Softmax normalization divides E'-scaled numerator by E'-scaled s: exact.

Engine budget per image (measured cost model): PE 224 MM slots ~58us,
ScalarE (all exp drains + q/xrb) ~55us, DVE (k/v drains, normalize,
recip, residual, casts) ~50us. PSUM: shared 2-bank work pool bufs=3
(6 banks) + AV accumulator (2 banks). Drains move >=1024 elem per op.
Emission is software-pipelined: scores(h+1) interleave with AV(h) so the
ScalarE exp-drain latency hides under AV/qk matmuls.
"""
import sys

try:
    import concourse.bass as bass  # noqa: F401
except ImportError:
    sys.path.insert(0, "/opt/trn_rl_repo")

from contextlib import ExitStack

import numpy as np

import concourse.bass as bass
import concourse.mybir as mybir
import concourse.tile as tile
from concourse import bacc
from concourse.bass_utils import run_bass_kernel_spmd

F32 = mybir.dt.float32
FP8 = mybir.dt.float8e4
EXP = mybir.ActivationFunctionType.Exp
IDENT = mybir.ActivationFunctionType.Identity
DR = mybir.MatmulPerfMode.DoubleRow
MUL = mybir.AluOpType.mult

B_PER_CORE = 2   # 16 images / 8 cores
C = 256          # channels == head dim
N = 1024         # tokens (32*32)
HEADS = 4
SCALE = C ** -0.5
E_BIAS = float(np.log(2.0 ** -6))  # exp pre-scale: E' = exp(s/16)*2^-6
N_CORES = 8


def _build():
    nc = bacc.Bacc("TRN2", debug=False, num_devices=N_CORES)
    x_d = nc.declare_dram_parameter("x", [B_PER_CORE, C, N], F32, isOutput=False)
    wp_d = nc.declare_dram_parameter("W_proj", [C, 3 * HEADS * C], F32, isOutput=False)
    bp_d = nc.declare_dram_parameter("b_proj", [3 * HEADS * C], F32, isOutput=False)
    wo_d = nc.declare_dram_parameter("W_out", [HEADS * C, C], F32, isOutput=False)
    bo_d = nc.declare_dram_parameter("b_out", [C], F32, isOutput=False)
    out_d = nc.declare_dram_parameter("out", [B_PER_CORE, C, N], F32, isOutput=True)

    with tile.TileContext(nc) as tc, ExitStack() as ctx:
        pool = ctx.enter_context(tc.tile_pool(name="persist", bufs=1))
        stage_pool = ctx.enter_context(tc.tile_pool(name="stage", bufs=3))
        xr_pool = ctx.enter_context(tc.tile_pool(name="xr", bufs=2))
        xb_pool = ctx.enter_context(tc.tile_pool(name="xb", bufs=2))
        v2_pool = ctx.enter_context(tc.tile_pool(name="v2", bufs=2))
        qk_pool = ctx.enter_context(tc.tile_pool(name="qk", bufs=2))
        e_pool = ctx.enter_context(tc.tile_pool(name="e", bufs=2))
        cat_pool = ctx.enter_context(tc.tile_pool(name="cat", bufs=2))
        r_pool = ctx.enter_context(tc.tile_pool(name="r", bufs=2))
        xrb_pool = ctx.enter_context(tc.tile_pool(name="xrb", bufs=2))
        out_pool = ctx.enter_context(tc.tile_pool(name="outs", bufs=2))
        ps_work = ctx.enter_context(tc.tile_pool(name="psw", bufs=3, space="PSUM"))
        ps_acc = ctx.enter_context(tc.tile_pool(name="psa", bufs=1, space="PSUM"))

        # ---- input DMAs + fp8 casts, first-needed data first ----
        xr_tiles = []
        xr = xr_pool.tile([128, 2, N], F32, tag="xr")
        for kt in range(2):
            for isl in range(2):
                nc.sync.dma_start(
                    out=xr[:, kt, isl * 512:(isl + 1) * 512],
                    in_=x_d[0, kt * 128:(kt + 1) * 128, isl * 512:(isl + 1) * 512])
        xr_tiles.append(xr)

        # W_proj: q,k cols (first 512 of each 768 block) -> wqk; v cols -> wv
        wqk = pool.tile([128, 2, 4 * 512], FP8)
        wv = pool.tile([128, 2, 4 * 256], FP8)
        b_sb = None
        for h in range(HEADS):
            for kt in range(2):
                ws = stage_pool.tile([128, 512], F32, tag="wstage")
                nc.sync.dma_start(
                    out=ws[:],
                    in_=wp_d[kt * 128:(kt + 1) * 128, h * 768:h * 768 + 512])
                nc.vector.tensor_copy(wqk[:, kt, h * 512:(h + 1) * 512], ws[:])
            if h == 0:
                # v weights for all heads (needed right after head-0 qk)
                for kt in range(2):
                    vs = stage_pool.tile([128, 4, 256], F32, tag="vstage")
                    nc.sync.dma_start(
                        out=vs[:],
                        in_=wp_d[kt * 128:(kt + 1) * 128, :].rearrange(
                            "p (h x) -> p h x", h=4)[:, :, 512:768])
                    nc.vector.tensor_copy(
                        wv[:, kt, :].rearrange("p (h x) -> p h x", h=4), vs[:])
                b_sb = pool.tile([128, 24], F32)  # b_proj, tile t
                nc.sync.dma_start(
                    out=b_sb[:], in_=bp_d[:].rearrange("(t p) -> p t", p=128))
                bo_sb = pool.tile([128, 2], F32)
                nc.sync.dma_start(out=bo_sb[:],
                                  in_=bo_d[:].rearrange("(t p) -> p t", p=128))

        # second image's x: queued last, prefetched during image-0 compute
        xr = xr_pool.tile([128, 2, N], F32, tag="xr")
        for kt in range(2):
            nc.sync.dma_start(out=xr[:, kt, :],
                              in_=x_d[1, kt * 128:(kt + 1) * 128, :])
        xr_tiles.append(xr)

        # ---- small constants ----
        ones8 = pool.tile([128, 2, 128], FP8)
        nc.vector.memset(ones8[:], 1.0)
        eb_sb = pool.tile([128, 1], F32)  # exp bias: ln(2^-6)
        nc.vector.memset(eb_sb[:], E_BIAS)
        wrm = pool.tile([128, 2, 512], FP8)
        nc.vector.memset(wrm[:], 1.0)

        # dummy DR matmuls: fill the initial DMA wait + warm the HAM clock gate
        for wi in range(16):
            warm_ps = ps_work.tile([128, 2, 512], F32, tag="work")
            nc.tensor.matmul(out=warm_ps[:, wi % 2, :], lhsT=ones8[:],
                             rhs=wrm[:], start=True, stop=True, perf_mode=DR)

        total_bias = pool.tile([128, 2], F32)
        wo_sb = pool.tile([128, 8, 256], FP8)  # W_out kt-tiles (loaded early img 0)
        zb = pool.tile([128, 8, 2], FP8)

        def emit_qk(h, qk_t, xb):
            """q,k for head h -> qk_t[128, 4(q0 q1 k0 k1), N] fp8."""
            for mt in range(4):
                ps = ps_work.tile([128, 2, 512], F32, tag="work")
                lhs = wqk[:, :, h * 512 + mt * 128:h * 512 + (mt + 1) * 128]
                for isl in range(2):
                    nc.tensor.matmul(
                        out=ps[:, isl, :], lhsT=lhs,
                        rhs=xb[:, :, isl * 512:(isl + 1) * 512],
                        start=True, stop=True, perf_mode=DR)
                col = h * 6 + mt
                dest = qk_t[:, mt, :].rearrange("p (a x) -> p a x", a=2)
                if mt < 2:  # q rows: ScalarE
                    nc.scalar.activation(dest, ps[:], IDENT,
                                         bias=b_sb[:, col:col + 1])
                else:       # k rows: DVE
                    nc.vector.tensor_scalar_add(dest, ps[:], b_sb[:, col:col + 1])

        def emit_v(hp, v2, xb):
            """v for heads 2hp, 2hp+1 -> v2[:, it, h*256+d] (natural layout)."""
            for itp in range(4):
                ps = ps_work.tile([128, 2, 512], F32, tag="work")
                for j in range(2):
                    it = 2 * itp + j
                    nc.tensor.matmul(
                        out=ps[:, j, :],
                        lhsT=xb[:, :, it * 128:(it + 1) * 128],
                        rhs=wv[:, :, hp * 512:(hp + 1) * 512],
                        start=True, stop=True, perf_mode=DR)
                nc.vector.tensor_copy(
                    v2[:, 2 * itp:2 * itp + 2, hp * 512:(hp + 1) * 512], ps[:])

        def emit_scores(h, qk_t, e_t, isl):
            """attT jt-pair tiles -> E' = exp(attT/16)*2^-6 in fp8."""
            for a in range(4):
                ps = ps_work.tile([128, 2, 512], F32, tag="work")
                for j in range(2):
                    jt = 2 * a + j
                    nc.tensor.matmul(
                        out=ps[:, j, :],
                        lhsT=qk_t[:, 2:4, jt * 128:(jt + 1) * 128],
                        rhs=qk_t[:, 0:2, isl * 512:(isl + 1) * 512],
                        start=True, stop=True, perf_mode=DR)
                nc.scalar.activation(
                    e_t[:, 2 * a:2 * a + 2, isl * 512:(isl + 1) * 512],
                    ps[:], EXP, scale=SCALE, bias=eb_sb[:])

        def emit_av(h, e_t, v2, cat, isl):
            """denominator + AV for one i-half; normalized into catT (fp8)."""
            s_t = ps_work.tile([128, 2, 512], F32, tag="work")
            s_ps = s_t[:, 0, :]
            for a in range(4):
                nc.tensor.matmul(
                    out=s_ps, lhsT=ones8[:],
                    rhs=e_t[:, 2 * a:2 * a + 2, isl * 512:(isl + 1) * 512],
                    perf_mode=DR, start=(a == 0), stop=(a == 3))
            o_ps = ps_acc.tile([128, 2, 512], F32, tag="acc")
            for a in range(4):
                e_ap = e_t[:, 2 * a:2 * a + 2, isl * 512:(isl + 1) * 512]
                for dh in range(2):
                    nc.tensor.matmul(
                        out=o_ps[:, dh, :],
                        lhsT=v2[:, 2 * a:2 * a + 2,
                                h * 256 + dh * 128:h * 256 + (dh + 1) * 128],
                        rhs=e_ap, start=(a == 0), stop=(a == 3), perf_mode=DR)
            r_sb = r_pool.tile([128, 512], F32, tag="r")
            nc.vector.reciprocal_approx_fast(r_sb[:], s_ps)
            for dh in range(2):
                nc.vector.scalar_tensor_tensor(
                    cat[:, 2 * h + dh, isl * 512:(isl + 1) * 512],
                    o_ps[:, dh, :], 1.0, r_sb[:], MUL, MUL)

        for b in range(B_PER_CORE):
            xr = xr_tiles[b]
            xb = xb_pool.tile([128, 2, N], FP8, tag="xb")
            for kt in range(2):
                nc.vector.tensor_copy(xb[:, kt, :], xr[:, kt, :])
            v2 = v2_pool.tile([128, 8, 1024], FP8, tag="v2")
            cat = cat_pool.tile([128, 8, N], FP8, tag="cat")

            qk_t = {0: qk_pool.tile([128, 4, N], FP8, tag="qk", name="qk_t")}
            emit_qk(0, qk_t[0], xb)
            emit_v(0, v2, xb)
            e_tt = {0: e_pool.tile([128, 8, N], FP8, tag="e", name="e_t")}
            emit_scores(0, qk_t[0], e_tt[0], 0)
            emit_scores(0, qk_t[0], e_tt[0], 1)

            for h in range(HEADS):
                if h + 1 < HEADS:
                    qk_t[h + 1] = qk_pool.tile([128, 4, N], FP8, tag="qk",
                                               name="qk_t")
                    emit_qk(h + 1, qk_t[h + 1], xb)
                if h == 1:
                    emit_v(1, v2, xb)
                if b == 0 and h == 0:
                    # W_out + deferred-bias setup, overlapped with early compute
                    wos = stage_pool.tile([128, 8, 256], F32, tag="wostage")
                    nc.sync.dma_start(
                        out=wos[:],
                        in_=wo_d[:, :].rearrange("(t p) c -> p t c", p=128))
                    nc.vector.tensor_copy(wo_sb[:], wos[:])
                    zscr = stage_pool.tile([128, 16], F32, tag="zscr")
                    nc.vector.memset(zscr[:], 0.0)
                    nc.vector.tensor_copy(
                        zb[:], zscr[:].rearrange("p (a c) -> p a c", c=2))
                    for kt in range(8):
                        hh, dt = kt // 2, kt % 2
                        nc.vector.tensor_copy(
                            zb[:, kt, 0:1],
                            b_sb[:, hh * 6 + 4 + dt:hh * 6 + 5 + dt])
                emit_av(h, e_tt[h], v2, cat, 0)
                if h + 1 < HEADS:
                    e_tt[h + 1] = e_pool.tile([128, 8, N], FP8, tag="e",
                                              name="e_t")
                    emit_scores(h + 1, qk_t[h + 1], e_tt[h + 1], 0)
                emit_av(h, e_tt[h], v2, cat, 1)
                if h + 1 < HEADS:
                    emit_scores(h + 1, qk_t[h + 1], e_tt[h + 1], 1)

            if b == 0:
                # b_v folds through softmax (weights sum to 1) and W_out:
                # total_bias[c] = b_out[c] + sum_hd b_v[hd] * W_out[hd, c].
                for ct in range(2):
                    bias_ps = ps_work.tile([128, 2], F32, tag="work")
                    for kt in range(8):
                        nc.tensor.matmul(out=bias_ps[:],
                                         lhsT=wo_sb[:, kt, ct * 128:(ct + 1) * 128],
                                         rhs=zb[:, kt, :],
                                         start=(kt == 0), stop=(kt == 7))
                    nc.vector.tensor_add(total_bias[:, ct:ct + 1], bias_ps[:, 0:1],
                                         bo_sb[:, ct:ct + 1])

            # residual + bias, broadcast along tokens: xrb = x_r + total_bias
            xrb = xrb_pool.tile([128, 2, 2, 512], F32, tag="xrb")
            for ct in range(2):
                nc.scalar.activation(
                    xrb[:, ct],
                    xr[:, ct, :].rearrange("p (a x) -> p a x", a=2),
                    IDENT, bias=total_bias[:, ct:ct + 1])

            # ---- out projection + residual, already in output layout ----
            for ct in range(2):
                res_ps = ps_work.tile([128, 2, 512], F32, tag="work")
                for isl in range(2):
                    for t in range(4):
                        nc.tensor.matmul(
                            out=res_ps[:, isl, :],
                            lhsT=wo_sb[:, 2 * t:2 * t + 2, ct * 128:(ct + 1) * 128],
                            rhs=cat[:, 2 * t:2 * t + 2, isl * 512:(isl + 1) * 512],
                            start=(t == 0), stop=(t == 3), perf_mode=DR)
                o_sb = out_pool.tile([128, 2, 512], F32, tag="o_sb")
                nc.vector.tensor_add(o_sb[:], res_ps[:], xrb[:, ct])
                nc.sync.dma_start(
                    out=out_d[b, ct * 128:(ct + 1) * 128, :].rearrange(
                        "p (a x) -> p a x", a=2),
                    in_=o_sb[:])

    nc.compile()
    return nc


_NC = None


def kernel(x, W_proj, b_proj, W_out, b_out):
    global _NC
    if _NC is None:
        _NC = _build()
    x = np.ascontiguousarray(x, dtype=np.float32).reshape(16, C, N)
    in_maps = [
        {
            "x": x[i * B_PER_CORE:(i + 1) * B_PER_CORE],
            "W_proj": np.ascontiguousarray(W_proj, dtype=np.float32),
            "b_proj": np.ascontiguousarray(b_proj, dtype=np.float32),
            "W_out": np.ascontiguousarray(W_out, dtype=np.float32),
            "b_out": np.ascontiguousarray(b_out, dtype=np.float32),
        }
        for i in range(N_CORES)
    ]
    res = run_bass_kernel_spmd(_NC, in_maps, core_ids=list(range(N_CORES)))
    out = np.concatenate([res.results[i]["out"] for i in range(N_CORES)], axis=0)
    return out.reshape(16, C, 32, 32)
